# revision 12
# baseline (speedup 1.0000x reference)
"""GCN + pathway model on 8 TRN2 cores — node-sharded full-batch variant.

Each core owns every 8th node (striped by global degree rank) for ALL 8
batch elements; gather rows pack the whole batch (512 values -> 1KB bf16
descriptors), cutting SWDGE descriptor generation 8x vs batch-sharding.
Per layer: local transform -> bf16 AllGather of the message table ->
prefix-round gathers (local dst prefix, ZROW-padded) -> dense epilogue.
Pathway pooling is a dense PE matmul with a host-built multiplicity
matrix; the MLP head computes all 8 batches at once.
"""

import numpy as np

BS, N, F, H, L = 8, 15135, 64, 64, 3
NCMT, HFC, NCLS = 400, 200, 2
P = 128
NC8 = 8
CNL = 15                    # local node slots per core
NL = CNL * P                # 1920 local nodes per core
NPAD = NL * NC8             # 15360
FB = BS * F                 # 512 features x batches per node row
ZROW_G = NPAD - 1           # global pad node (on core 7) with zero row
NCMT_SLOTS = 4              # 512 padded pathways
SG = 8                      # gather group: 8 slots = 1024 idx = 1MB bf16
HB = HFC // 2               # 100


def blocked(r):
    """global new-label node id -> AllGather-blocked table row."""
    return (r % NC8) * NL + r // NC8


def host_prep(edge_index, row, col, fc_w, fc_b, lin1_w, lin1_b):
    src0 = edge_index[0].astype(np.int64)
    dst0 = edge_index[1].astype(np.int64)
    loop = np.arange(N, dtype=np.int64)
    # self-loops FIRST so each node's slot-0 edge is the identity
    src = np.concatenate([loop, src0])
    dst = np.concatenate([loop, dst0])
    deg = np.bincount(dst, minlength=N)
    dinv = np.zeros(N, np.float32)
    dinv[deg > 0] = (1.0 / np.sqrt(deg[deg > 0].astype(np.float32))).astype(np.float32)

    perm = np.argsort(-deg, kind="stable")  # new -> old
    inv = np.empty(N, np.int64)
    inv[perm] = np.arange(N)
    src_n = inv[src]
    dst_n = inv[dst]
    degs = deg[perm]
    order = np.argsort(dst_n, kind="stable")
    src_by_dst = src_n[order]  # per dst (new label), self-loop first

    # per-core local CSR (nodes striped: new label r -> core r%8, local j=r//8)
    deg_pad = np.zeros(NPAD, np.int64)
    deg_pad[:N] = degs
    starts = np.zeros(N + 1, np.int64)
    starts[1:] = np.cumsum(degs)

    maxdeg = int(degs[0])
    # rounds k=1..maxdeg-1 (k=0 handled densely as the self-loop add)
    # shared structure: ck = ceil(max_c n_k_c / 128)
    cks = []
    nkc_all = []
    for k in range(1, maxdeg):
        nkc = []
        for c in range(NC8):
            dl = deg_pad[c::NC8][:NL]  # local degs (descending)
            nkc.append(int(np.searchsorted(-dl, -k, side="left")))
        nkc_all.append(nkc)
        cks.append((max(nkc) + P - 1) // P)
    # build per-core streams; idx values are blocked global labels
    streams = [[] for _ in range(NC8)]
    slot_meta = []  # (round k, accslot)
    for ki, k in enumerate(range(1, maxdeg)):
        ck = cks[ki]
        if ck == 0:
            continue
        for c in range(NC8):
            n_k = nkc_all[ki][c]
            # local nodes 0..n_k-1 are global labels r = 8*j + c
            js = np.arange(n_k, dtype=np.int64)
            rs = js * NC8 + c
            idx_k = src_by_dst[starts[rs] + k]  # k-th in-edge src (global new)
            bk = (blocked(idx_k)).astype(np.int64)
            padn = ck * P - n_k
            if padn:
                bk = np.concatenate([bk, np.full(padn, ZROW_G, np.int64)])
            streams[c].append(bk)
        slot_meta.extend((k, s) for s in range(ck))
    n_slots = len(slot_meta)
    n_slots_pad = ((n_slots + SG - 1) // SG) * SG
    pad_slots = n_slots_pad - n_slots
    for c in range(NC8):
        if pad_slots:
            streams[c].append(np.full(pad_slots * P, ZROW_G, np.int64))
    idx16 = []
    for c in range(NC8):
        st = np.concatenate(streams[c]).astype(np.int32)
        a16 = st.astype(np.int16).reshape(-1, 16).T
        idx16.append(np.tile(a16, (8, 1)))  # [128, T/16]

    # groups of SG slots -> contiguous add-runs (shared across cores)
    slot_round = np.array([m[0] for m in slot_meta], np.int32)
    slot_accslot = np.array([m[1] for m in slot_meta], np.int32)
    groups = []
    for g0 in range(0, n_slots_pad, SG):
        g1 = min(g0 + SG, n_slots)
        runs = []
        j = g0
        while j < g1:
            k0 = slot_round[j]
            a0 = slot_accslot[j]
            ln = 1
            while (
                j + ln < g1
                and slot_round[j + ln] == k0
                and slot_accslot[j + ln] == a0 + ln
            ):
                ln += 1
            runs.append((j - g0, int(a0), ln))
            j += ln
        groups.append(runs)

    # dinv local [128, CNL] per core; x_local [NL, FB] per core
    dinv_n = np.zeros(NPAD, np.float32)
    dinv_n[:N] = dinv[perm]
    dinv_loc = []
    for c in range(NC8):
        dl = dinv_n[c::NC8][:NL]
        dinv_loc.append(dl.reshape(CNL, P).T.copy())

    # pathway multiplicity matrix A [NPAD(blocked), 512] (counts)
    r_n = inv[row.astype(np.int64)]
    A = np.zeros((NPAD, NCMT_SLOTS * P), np.float32)
    np.add.at(A, (blocked(r_n), col.astype(np.int64)), 1.0)

    u = [np.ascontiguousarray(fc_w[0, l::L]).astype(np.float32) for l in range(L)]
    pcnt = np.bincount(col.astype(np.int64), minlength=NCMT)
    cnt_clip = np.maximum(pcnt, 1).astype(np.float32)
    lin1_eff = (lin1_w / cnt_clip[None, :]).astype(np.float32)
    lin1_b_eff = (lin1_b + fc_b[0] * lin1_w.sum(axis=1)).astype(np.float32)

    return dict(
        perm=perm,
        inv=inv,
        dinv_loc=dinv_loc,
        dinv_n=dinv_n,
        idx16=idx16,
        groups=groups,
        A=A,
        u=u,
        lin1_eff=lin1_eff,
        lin1_b_eff=lin1_b_eff,
    )


def golden(inputs, prep, bf16=True):
    """Numpy simulation of the device pipeline (all cores), full batch."""
    import ml_dtypes

    def tobf(a):
        return a.astype(ml_dtypes.bfloat16).astype(np.float32) if bf16 else a

    x = np.asarray(inputs["x"], np.float32)  # [BS, N, F]
    perm = prep["perm"]
    dinv_n = prep["dinv_n"]  # [NPAD] new labels
    # h in new-label order, [NPAD, BS, F]
    h = np.zeros((NPAD, BS, F), np.float32)
    h[:N] = x[:, perm, :].transpose(1, 0, 2)
    Ws = [inputs["W1"], inputs["W2"], inputs["W3"]]
    bs_ = [inputs["b1"], inputs["b2"], inputs["b3"]]
    qacc = np.zeros((NPAD, BS), np.float32)
    for l in range(3):
        hp = h * dinv_n[:, None, None]
        m = np.einsum("nbf,hf->nbh", hp, Ws[l])  # [NPAD, BS, H]
        table = tobf(m)  # AllGather in bf16
        # blocked table order
        tbl = np.zeros_like(table)
        rr = np.arange(NPAD)
        tbl[blocked(rr)] = table
        acc = m.copy()  # self-loop round (local fp32 add)
        # streamed rounds per core
        for c in range(NC8):
            st = prep["idx16"][c][:16].T.reshape(-1).astype(np.int64)
            for g, runs in enumerate(prep["groups"]):
                base = g * SG * P
                G = tbl[st[base : base + SG * P]].reshape(SG, P, BS, F)
                for (ss, a0, ln) in runs:
                    # local acc rows of core c: global r = 8*j + c
                    for t in range(ln):
                        js = (a0 + t) * P + np.arange(P)
                        rs = js * NC8 + c
                        acc[rs] += G[ss + t]
        h = np.maximum(acc * dinv_n[:, None, None] + bs_[l][None, None, :], 0.0)
        qacc += h @ prep["u"][l]
    # pathway: s[c,b] = sum_r A[blocked(r), c] * q[r, b]
    rr = np.arange(NPAD)
    qb = np.zeros_like(qacc)
    qb[blocked(rr)] = qacc
    s = prep["A"].T @ tobf(qb)  # [512, BS]
    s = s[:NCMT]
    z1 = np.maximum(prep["lin1_eff"] @ s + prep["lin1_b_eff"][:, None], 0.0)
    z2 = (np.asarray(inputs["lin2_w"]) @ z1 + np.asarray(inputs["lin2_b"])[:, None]).T
    mx = z2.max(-1, keepdims=True)
    t = z2 - mx
    return t - np.log(np.exp(t).sum(-1, keepdims=True))


def build_bass(prep):
    import concourse.bacc as bacc
    import concourse.mybir as mybir
    import concourse.tile as tile
    from concourse.masks import make_identity

    f32 = mybir.dt.float32
    bf16 = mybir.dt.bfloat16
    f8 = mybir.dt.float8e4
    i16 = mybir.dt.int16
    AF = mybir.ActivationFunctionType
    ALU = mybir.AluOpType
    AX = mybir.AxisListType

    groups = prep["groups"]
    Lm = prep["idx16"][0].shape[1]

    nc = bacc.Bacc("TRN2", target_bir_lowering=False, debug=False, num_devices=NC8)
    x_in = nc.dram_tensor("x", [NL, FB], f32, kind="ExternalInput")
    idx_in = nc.dram_tensor("idx16", [P, Lm], i16, kind="ExternalInput")
    dinv_in = nc.dram_tensor("dinv", [P, CNL], f32, kind="ExternalInput")
    wtab_in = nc.dram_tensor("wtab", [3, 64, 64], f32, kind="ExternalInput")
    btab_in = nc.dram_tensor("btab", [3, P, 64], f32, kind="ExternalInput")
    utab_in = nc.dram_tensor("utab", [3, P, 64], f32, kind="ExternalInput")
    a_in = nc.dram_tensor("amat", [NPAD, NCMT_SLOTS * P], bf16, kind="ExternalInput")
    lin1t_in = nc.dram_tensor("lin1t", [4 * P, HFC], f32, kind="ExternalInput")
    lin1b_in = nc.dram_tensor("lin1b", [HB, 2], f32, kind="ExternalInput")
    lin2t_in = nc.dram_tensor("lin2t", [HB, 2 * NCLS], f32, kind="ExternalInput")
    lin2b_in = nc.dram_tensor("lin2b", [BS, NCLS], f32, kind="ExternalInput")
    out_t = nc.dram_tensor("out", [BS, NCLS], f32, kind="ExternalOutput")
    ag_in = nc.dram_tensor("ag_in", [NL, FB], f8, kind="Internal")
    ag_out = nc.dram_tensor("ag_out", [NPAD, FB], f8, kind="Internal", addr_space="Shared")
    qag_in = nc.dram_tensor("qag_in", [NL, BS], f32, kind="Internal")
    qag_out = nc.dram_tensor("qag_out", [NPAD, BS], f32, kind="Internal", addr_space="Shared")

    RG = [list(range(NC8))]
    SGA = 8  # A-matrix staging slots

    with tile.TileContext(nc) as tc:
        with (
            tc.tile_pool(name="main", bufs=1) as pool,
            tc.tile_pool(name="psum", bufs=2, space="PSUM") as pp,
        ):
            h = pool.tile([P, CNL * FB], f32, tag="h")
            m_sb = pool.tile([P, CNL * FB], f32, tag="m_sb")
            acc = pool.tile([P, CNL * FB], f32, tag="acc")
            m_bf = pool.tile([P, CNL * FB], f8, tag="m_bf")
            gbuf = [pool.tile([P, SG * FB], f8, tag=f"g{i}", name=f"g{i}") for i in range(2)]
            abuf = [pool.tile([P, SGA * 512], bf16, tag=f"a{i}", name=f"a{i}") for i in range(2)]
            idx_sb = pool.tile([P, Lm], i16, tag="idx_sb")
            dinv_sb = pool.tile([P, CNL], f32, tag="dinv_sb")
            wt_sb = pool.tile([64, 3 * 64], f32, tag="wt_sb")
            b_sb = pool.tile([P, 3 * 64], f32, tag="b_sb")
            u_sb = pool.tile([P, 3 * 64], f32, tag="u_sb")
            ident = pool.tile([P, P], f32, tag="ident")
            qacc = pool.tile([P, CNL * BS], f32, tag="qacc")
            qc = pool.tile([P, CNL * BS], f32, tag="qc")
            qfull = pool.tile([P, (NPAD // P) * BS], f32, tag="qfull")
            qfull_bf = pool.tile([P, (NPAD // P) * BS], bf16, tag="qfull_bf")
            s_sb = pool.tile([P, NCMT_SLOTS * BS], f32, tag="s_sb")
            lin1t_sb = pool.tile([P, 4 * 2 * HB], f32, tag="lin1t_sb")
            lin1b_sb = pool.tile([HB, 2], f32, tag="lin1b_sb")
            lin2t_sb = pool.tile([HB, 2 * NCLS], f32, tag="lin2t_sb")
            lin2b_sb = pool.tile([BS, NCLS], f32, tag="lin2b_sb")
            z1_sb = pool.tile([HB, 2 * BS], f32, tag="z1_sb")
            z2_sb = pool.tile([BS, NCLS], f32, tag="z2_sb")
            sm_t = pool.tile([BS, NCLS], f32, tag="sm_t")
            sm_e = pool.tile([BS, NCLS], f32, tag="sm_e")
            sm_r = pool.tile([BS, 2], f32, tag="sm_r")

            nc.sync.dma_start(idx_sb[:], idx_in[:])
            nc.sync.dma_start(dinv_sb[:], dinv_in[:])
            for l in range(3):
                nc.sync.dma_start(wt_sb[:, l * 64 : (l + 1) * 64], wtab_in[l])
                nc.sync.dma_start(b_sb[:, l * 64 : (l + 1) * 64], btab_in[l])
                nc.sync.dma_start(u_sb[:, l * 64 : (l + 1) * 64], utab_in[l])
            nc.sync.dma_start(
                lin1t_sb[:].rearrange("p (k m) -> p k m", k=4),
                lin1t_in[:].rearrange("(k p) m -> p k m", p=P),
            )
            nc.sync.dma_start(lin1b_sb[:], lin1b_in[:])
            nc.sync.dma_start(lin2t_sb[:], lin2t_in[:])
            nc.sync.dma_start(lin2b_sb[:], lin2b_in[:])
            make_identity(nc, ident[:])

            h4 = h[:].rearrange("p (s b f) -> p s b f", b=BS, f=F)
            m4 = m_sb[:].rearrange("p (s b f) -> p s b f", b=BS, f=F)
            dinv_b = (
                dinv_sb[:].rearrange("p s -> p s () ()").to_broadcast([P, CNL, BS, F])
            )
            # x straight into h (host packs the local layout)
            nc.sync.dma_start(h4, x_in[:].rearrange("(s p) f -> p s f", p=P))

            for l in range(3):
                bl = b_sb[:, l * 64 : (l + 1) * 64]
                ul = u_sb[:, l * 64 : (l + 1) * 64]
                nc.vector.tensor_tensor(out=h4, in0=h4, in1=dinv_b, op=ALU.mult)
                for s in range(CNL):
                    for b in range(BS):
                        pt = pp.tile([64, P], f32, tag="pt")
                        nc.tensor.transpose(pt[:], h4[:, s, b, :], ident[:])
                        ht = pool.tile([64, P], f32, tag="ht")
                        nc.vector.tensor_copy(ht[:], pt[:])
                        pm = pp.tile([P, F], f32, tag="pm")
                        nc.tensor.matmul(
                            pm[:],
                            lhsT=ht[:],
                            rhs=wt_sb[:, l * 64 : (l + 1) * 64],
                            start=True,
                            stop=True,
                        )
                        nc.vector.tensor_copy(m4[:, s, b, :], pm[:])
                nc.vector.tensor_copy(m_bf[:], m_sb[:])
                nc.sync.dma_start(
                    ag_in[:].rearrange("(s p) f -> p s f", p=P),
                    m_bf[:].rearrange("p (s f) -> p s f", f=FB),
                )
                nc.gpsimd.collective_compute(
                    "AllGather",
                    ALU.bypass,
                    replica_groups=RG,
                    ins=[ag_in[:]],
                    outs=[ag_out[:]],
                )
                # self-loop round: acc = m (local, fp32)
                nc.vector.tensor_copy(acc[:], m_sb[:])
                for g, runs in enumerate(groups):
                    gt = gbuf[g % 2]
                    nc.gpsimd.dma_gather(
                        out_ap=gt[:].rearrange("p (s f) -> p s f", f=FB),
                        in_ap=ag_out[:],
                        idxs_ap=idx_sb[:, g * SG * 8 : (g + 1) * SG * 8],
                        num_idxs=SG * P,
                        num_idxs_reg=SG * P,
                        elem_size=FB,
                        single_packet=False,
                    )
                    for (ss, a0, ln) in runs:
                        nc.vector.tensor_tensor(
                            out=acc[:, a0 * FB : (a0 + ln) * FB],
                            in0=acc[:, a0 * FB : (a0 + ln) * FB],
                            in1=gt[:, ss * FB : (ss + ln) * FB],
                            op=ALU.add,
                        )
                acc4 = acc[:].rearrange("p (s b f) -> p s b f", b=BS, f=F)
                nc.vector.tensor_tensor(out=h4, in0=acc4, in1=dinv_b, op=ALU.mult)
                nc.vector.tensor_tensor(
                    out=h4,
                    in0=h4,
                    in1=bl.rearrange("p f -> p () () f").to_broadcast([P, CNL, BS, F]),
                    op=ALU.add,
                )
                nc.scalar.activation(h[:], h[:], AF.Relu)
                nc.vector.tensor_tensor(
                    out=m4,
                    in0=h4,
                    in1=ul.rearrange("p f -> p () () f").to_broadcast([P, CNL, BS, F]),
                    op=ALU.mult,
                )
                if l == 0:
                    nc.vector.reduce_sum(qacc[:], m4, axis=AX.X)
                else:
                    nc.vector.reduce_sum(qc[:], m4, axis=AX.X)
                    nc.vector.tensor_tensor(
                        out=qacc[:], in0=qacc[:], in1=qc[:], op=ALU.add
                    )

            # ---- pathway: s = A^T q via PE ----
            nc.sync.dma_start(
                qag_in[:].rearrange("(s p) b -> p s b", p=P),
                qacc[:].rearrange("p (s b) -> p s b", b=BS),
            )
            nc.gpsimd.collective_compute(
                "AllGather", ALU.bypass, replica_groups=RG,
                ins=[qag_in[:]], outs=[qag_out[:]],
            )
            nc.sync.dma_start(
                qfull[:].rearrange("p (s b) -> p s b", b=BS),
                qag_out[:].rearrange("(s p) b -> p s b", p=P),
            )
            nc.vector.tensor_copy(qfull_bf[:], qfull[:])
            qf3 = qfull_bf[:].rearrange("p (s b) -> p s b", b=BS)
            a_view = a_in[:].rearrange("(s p) c -> p s c", p=P)
            NSLOT = NPAD // P  # 120
            s_ps_all = pp.tile([P, 4 * BS], f32, tag="s_ps", bufs=1, name="s_ps_all")
            s_ps = [s_ps_all[:, g * BS : (g + 1) * BS] for g in range(4)]
            for sc in range(NSLOT):
                if sc % SGA == 0:
                    ab = abuf[(sc // SGA) % 2]
                    nc.sync.dma_start(
                        ab[:].rearrange("p (k c) -> p k c", k=SGA),
                        a_view[:, sc : sc + SGA, :],
                    )
                ab = abuf[(sc // SGA) % 2]
                a3 = ab[:].rearrange("p (k c) -> p k c", k=SGA)
                for gp in range(4):
                    nc.tensor.matmul(
                        s_ps[gp],
                        lhsT=a3[:, sc % SGA, gp * P : (gp + 1) * P],
                        rhs=qf3[:, sc, :],
                        start=(sc == 0),
                        stop=(sc == NSLOT - 1),
                        skip_group_check=True,
                    )
            s3 = s_sb[:].rearrange("p (k b) -> p k b", b=BS)
            for gp in range(4):
                nc.vector.tensor_copy(s3[:, gp, :], s_ps[gp])

            # ---- head (all 8 batches at once) ----
            for j, m0 in enumerate((0, HB)):
                pz = pp.tile([HB, BS], f32, tag="pz", bufs=1)
                for kc in range(4):
                    nc.tensor.matmul(
                        pz[:],
                        lhsT=lin1t_sb[:].rearrange("p (k m) -> p k m", k=4)[
                            :, kc, m0 : m0 + HB
                        ],
                        rhs=s3[:, kc, :],
                        start=(kc == 0),
                        stop=(kc == 3),
                    )
                nc.scalar.activation(
                    z1_sb[:].rearrange("p (j b) -> p j b", b=BS)[:, j, :],
                    pz[:],
                    AF.Relu,
                    bias=lin1b_sb[:, j : j + 1],
                )
            pz2 = pp.tile([BS, NCLS], f32, tag="pz2", bufs=1)
            z13 = z1_sb[:].rearrange("p (j b) -> p j b", b=BS)
            for j in range(2):
                nc.tensor.matmul(
                    pz2[:],
                    lhsT=z13[:, j, :],
                    rhs=lin2t_sb[:, j * NCLS : (j + 1) * NCLS],
                    start=(j == 0),
                    stop=(j == 1),
                )
            nc.vector.tensor_tensor(out=z2_sb[:], in0=pz2[:], in1=lin2b_sb[:], op=ALU.add)
            nc.vector.reduce_max(sm_r[:, 0:1], z2_sb[:], axis=AX.X)
            nc.vector.tensor_tensor(
                out=sm_t[:],
                in0=z2_sb[:],
                in1=sm_r[:, 0:1].to_broadcast([BS, NCLS]),
                op=ALU.subtract,
            )
            nc.scalar.activation(sm_e[:], sm_t[:], AF.Exp)
            nc.vector.reduce_sum(sm_r[:, 1:2], sm_e[:], axis=AX.X)
            nc.scalar.activation(sm_r[:, 1:2], sm_r[:, 1:2], AF.Ln)
            nc.vector.tensor_tensor(
                out=sm_t[:],
                in0=sm_t[:],
                in1=sm_r[:, 1:2].to_broadcast([BS, NCLS]),
                op=ALU.subtract,
            )
            nc.sync.dma_start(out_t[:], sm_t[:])

    nc.compile()
    return nc


def _make_in_maps(inputs, prep):
    import ml_dtypes

    x = np.asarray(inputs["x"], np.float32)
    perm = prep["perm"]
    wtab = np.stack(
        [np.asarray(inputs[f"W{i}"], np.float32).T.copy() for i in (1, 2, 3)]
    )
    btab = np.stack(
        [np.tile(np.asarray(inputs[f"b{i}"], np.float32), (P, 1)) for i in (1, 2, 3)]
    )
    utab = np.stack([np.tile(u, (P, 1)) for u in prep["u"]])
    amat = prep["A"].astype(ml_dtypes.bfloat16)
    lin1t = np.zeros((4 * P, HFC), np.float32)
    lin1t[:NCMT] = prep["lin1_eff"].T
    lin1b = prep["lin1_b_eff"].reshape(2, HB).T.copy()
    lin2 = np.asarray(inputs["lin2_w"], np.float32)
    lin2t = np.concatenate([lin2.T[:HB], lin2.T[HB:]], axis=1)  # [100, 4]
    lin2b = np.tile(np.asarray(inputs["lin2_b"], np.float32).reshape(1, NCLS), (BS, 1))
    shared = dict(
        wtab=wtab, btab=btab, utab=utab, amat=amat,
        lin1t=lin1t, lin1b=lin1b, lin2t=lin2t, lin2b=lin2b,
    )
    # per-core x_local [NL, FB] and dinv/idx
    perm_pad = np.zeros(NPAD, np.int64)
    perm_pad[:N] = perm
    valid = np.zeros(NPAD, np.float32)
    valid[:N] = 1.0
    maps = []
    for c in range(NC8):
        rs = np.arange(NL) * NC8 + c
        olds = perm_pad[rs]
        xl = x[:, olds, :].transpose(1, 0, 2).reshape(NL, FB).copy()
        xl *= valid[rs][:, None]
        maps.append(
            dict(
                shared,
                x=xl,
                idx16=prep["idx16"][c],
                dinv=prep["dinv_loc"][c],
            )
        )
    return maps


_TRACE = {"trace": False, "last_ns": None}


def kernel(**inputs):
    from concourse.bass_utils import run_bass_kernel_spmd

    prep = host_prep(
        np.asarray(inputs["edge_index"]),
        np.asarray(inputs["row"]),
        np.asarray(inputs["col"]),
        np.asarray(inputs["fc_w"]),
        np.asarray(inputs["fc_b"]),
        np.asarray(inputs["lin1_w"]),
        np.asarray(inputs["lin1_b"]),
    )
    nc = build_bass(prep)
    in_maps = _make_in_maps(inputs, prep)
    res = run_bass_kernel_spmd(
        nc, in_maps, core_ids=list(range(NC8)), trace=_TRACE["trace"]
    )
    _TRACE["last_ns"] = res.exec_time_ns
    return res.results[0]["out"].reshape(BS, NCLS).astype(np.float32)


if __name__ == "__main__":
    import reference

    inputs = {k: np.asarray(v) for k, v in reference.setup_inputs().items()}
    expected = np.asarray(reference.reference(**inputs))
    prep = host_prep(
        inputs["edge_index"], inputs["row"], inputs["col"],
        inputs["fc_w"], inputs["fc_b"], inputs["lin1_w"], inputs["lin1_b"],
    )
    for bf in (False, True):
        got = golden(inputs, prep, bf16=bf)
        err = np.abs(got - expected).max()
        rel = err / np.abs(expected).max()
        print(f"golden(bf16={bf}) abs {err:.3e} rel {rel:.3e}")


# revision 13
# speedup vs baseline: 1.0127x; 1.0127x over previous
"""GCN + pathway model on 8 TRN2 cores — node-sharded full-batch variant.

Each core owns every 8th node (striped by global degree rank) for ALL 8
batch elements; gather rows pack the whole batch (512 values -> 1KB bf16
descriptors), cutting SWDGE descriptor generation 8x vs batch-sharding.
Per layer: local transform -> bf16 AllGather of the message table ->
prefix-round gathers (local dst prefix, ZROW-padded) -> dense epilogue.
Pathway pooling is a dense PE matmul with a host-built multiplicity
matrix; the MLP head computes all 8 batches at once.
"""

import numpy as np

BS, N, F, H, L = 8, 15135, 64, 64, 3
NCMT, HFC, NCLS = 400, 200, 2
P = 128
NC8 = 8
CNL = 15                    # local node slots per core
NL = CNL * P                # 1920 local nodes per core
NPAD = NL * NC8             # 15360
FB = BS * F                 # 512 features x batches per node row
ZROW_G = NPAD - 1           # global pad node (on core 7) with zero row
NCMT_SLOTS = 4              # 512 padded pathways
SG = 8                      # gather group: 8 slots = 1024 idx = 1MB bf16
HB = HFC // 2               # 100


def blocked(r):
    """global new-label node id -> AllGather-blocked table row."""
    return (r % NC8) * NL + r // NC8


def host_prep(edge_index, row, col, fc_w, fc_b, lin1_w, lin1_b):
    src0 = edge_index[0].astype(np.int64)
    dst0 = edge_index[1].astype(np.int64)
    loop = np.arange(N, dtype=np.int64)
    # self-loops FIRST so each node's slot-0 edge is the identity
    src = np.concatenate([loop, src0])
    dst = np.concatenate([loop, dst0])
    deg = np.bincount(dst, minlength=N)
    dinv = np.zeros(N, np.float32)
    dinv[deg > 0] = (1.0 / np.sqrt(deg[deg > 0].astype(np.float32))).astype(np.float32)

    perm = np.argsort(-deg, kind="stable")  # new -> old
    inv = np.empty(N, np.int64)
    inv[perm] = np.arange(N)
    src_n = inv[src]
    dst_n = inv[dst]
    degs = deg[perm]
    order = np.argsort(dst_n, kind="stable")
    src_by_dst = src_n[order]  # per dst (new label), self-loop first

    # per-core local CSR (nodes striped: new label r -> core r%8, local j=r//8)
    deg_pad = np.zeros(NPAD, np.int64)
    deg_pad[:N] = degs
    starts = np.zeros(N + 1, np.int64)
    starts[1:] = np.cumsum(degs)

    maxdeg = int(degs[0])
    # rounds k=1..maxdeg-1 (k=0 handled densely as the self-loop add)
    # shared structure: ck = ceil(max_c n_k_c / 128)
    cks = []
    nkc_all = []
    for k in range(1, maxdeg):
        nkc = []
        for c in range(NC8):
            dl = deg_pad[c::NC8][:NL]  # local degs (descending)
            nkc.append(int(np.searchsorted(-dl, -k, side="left")))
        nkc_all.append(nkc)
        cks.append((max(nkc) + P - 1) // P)
    # build per-core streams; idx values are blocked global labels
    streams = [[] for _ in range(NC8)]
    slot_meta = []  # (round k, accslot)
    for ki, k in enumerate(range(1, maxdeg)):
        ck = cks[ki]
        if ck == 0:
            continue
        for c in range(NC8):
            n_k = nkc_all[ki][c]
            # local nodes 0..n_k-1 are global labels r = 8*j + c
            js = np.arange(n_k, dtype=np.int64)
            rs = js * NC8 + c
            idx_k = src_by_dst[starts[rs] + k]  # k-th in-edge src (global new)
            bk = (blocked(idx_k)).astype(np.int64)
            padn = ck * P - n_k
            if padn:
                bk = np.concatenate([bk, np.full(padn, ZROW_G, np.int64)])
            streams[c].append(bk)
        slot_meta.extend((k, s) for s in range(ck))
    n_slots = len(slot_meta)
    n_slots_pad = ((n_slots + SG - 1) // SG) * SG
    pad_slots = n_slots_pad - n_slots
    for c in range(NC8):
        if pad_slots:
            streams[c].append(np.full(pad_slots * P, ZROW_G, np.int64))
    idx16 = []
    for c in range(NC8):
        st = np.concatenate(streams[c]).astype(np.int32)
        a16 = st.astype(np.int16).reshape(-1, 16).T
        idx16.append(np.tile(a16, (8, 1)))  # [128, T/16]

    # groups of SG slots -> contiguous add-runs (shared across cores)
    slot_round = np.array([m[0] for m in slot_meta], np.int32)
    slot_accslot = np.array([m[1] for m in slot_meta], np.int32)
    groups = []
    for g0 in range(0, n_slots_pad, SG):
        g1 = min(g0 + SG, n_slots)
        runs = []
        j = g0
        while j < g1:
            k0 = slot_round[j]
            a0 = slot_accslot[j]
            ln = 1
            while (
                j + ln < g1
                and slot_round[j + ln] == k0
                and slot_accslot[j + ln] == a0 + ln
            ):
                ln += 1
            runs.append((j - g0, int(a0), ln))
            j += ln
        groups.append(runs)

    # dinv local [128, CNL] per core; x_local [NL, FB] per core
    dinv_n = np.zeros(NPAD, np.float32)
    dinv_n[:N] = dinv[perm]
    dinv_loc = []
    for c in range(NC8):
        dl = dinv_n[c::NC8][:NL]
        dinv_loc.append(dl.reshape(CNL, P).T.copy())

    # pathway multiplicity matrix A [NPAD(blocked), 512] (counts)
    r_n = inv[row.astype(np.int64)]
    A = np.zeros((NPAD, NCMT_SLOTS * P), np.float32)
    np.add.at(A, (blocked(r_n), col.astype(np.int64)), 1.0)

    u = [np.ascontiguousarray(fc_w[0, l::L]).astype(np.float32) for l in range(L)]
    pcnt = np.bincount(col.astype(np.int64), minlength=NCMT)
    cnt_clip = np.maximum(pcnt, 1).astype(np.float32)
    lin1_eff = (lin1_w / cnt_clip[None, :]).astype(np.float32)
    lin1_b_eff = (lin1_b + fc_b[0] * lin1_w.sum(axis=1)).astype(np.float32)

    return dict(
        perm=perm,
        inv=inv,
        dinv_loc=dinv_loc,
        dinv_n=dinv_n,
        idx16=idx16,
        groups=groups,
        A=A,
        u=u,
        lin1_eff=lin1_eff,
        lin1_b_eff=lin1_b_eff,
    )


def golden(inputs, prep, bf16=True):
    """Numpy simulation of the device pipeline (all cores), full batch."""
    import ml_dtypes

    def tobf(a):
        return a.astype(ml_dtypes.bfloat16).astype(np.float32) if bf16 else a

    x = np.asarray(inputs["x"], np.float32)  # [BS, N, F]
    perm = prep["perm"]
    dinv_n = prep["dinv_n"]  # [NPAD] new labels
    # h in new-label order, [NPAD, BS, F]
    h = np.zeros((NPAD, BS, F), np.float32)
    h[:N] = x[:, perm, :].transpose(1, 0, 2)
    Ws = [inputs["W1"], inputs["W2"], inputs["W3"]]
    bs_ = [inputs["b1"], inputs["b2"], inputs["b3"]]
    qacc = np.zeros((NPAD, BS), np.float32)
    for l in range(3):
        hp = h * dinv_n[:, None, None]
        m = np.einsum("nbf,hf->nbh", hp, Ws[l])  # [NPAD, BS, H]
        table = tobf(m)  # AllGather in bf16
        # blocked table order
        tbl = np.zeros_like(table)
        rr = np.arange(NPAD)
        tbl[blocked(rr)] = table
        acc = m.copy()  # self-loop round (local fp32 add)
        # streamed rounds per core
        for c in range(NC8):
            st = prep["idx16"][c][:16].T.reshape(-1).astype(np.int64)
            for g, runs in enumerate(prep["groups"]):
                base = g * SG * P
                G = tbl[st[base : base + SG * P]].reshape(SG, P, BS, F)
                for (ss, a0, ln) in runs:
                    # local acc rows of core c: global r = 8*j + c
                    for t in range(ln):
                        js = (a0 + t) * P + np.arange(P)
                        rs = js * NC8 + c
                        acc[rs] += G[ss + t]
        h = np.maximum(acc * dinv_n[:, None, None] + bs_[l][None, None, :], 0.0)
        qacc += h @ prep["u"][l]
    # pathway: s[c,b] = sum_r A[blocked(r), c] * q[r, b]
    rr = np.arange(NPAD)
    qb = np.zeros_like(qacc)
    qb[blocked(rr)] = qacc
    s = prep["A"].T @ tobf(qb)  # [512, BS]
    s = s[:NCMT]
    z1 = np.maximum(prep["lin1_eff"] @ s + prep["lin1_b_eff"][:, None], 0.0)
    z2 = (np.asarray(inputs["lin2_w"]) @ z1 + np.asarray(inputs["lin2_b"])[:, None]).T
    mx = z2.max(-1, keepdims=True)
    t = z2 - mx
    return t - np.log(np.exp(t).sum(-1, keepdims=True))


def build_bass(prep):
    import concourse.bacc as bacc
    import concourse.mybir as mybir
    import concourse.tile as tile
    from concourse.masks import make_identity

    f32 = mybir.dt.float32
    bf16 = mybir.dt.bfloat16
    f8 = mybir.dt.float8e4
    i16 = mybir.dt.int16
    AF = mybir.ActivationFunctionType
    ALU = mybir.AluOpType
    AX = mybir.AxisListType

    groups = prep["groups"]
    Lm = prep["idx16"][0].shape[1]

    nc = bacc.Bacc("TRN2", target_bir_lowering=False, debug=False, num_devices=NC8)
    x_in = nc.dram_tensor("x", [NL, FB], f32, kind="ExternalInput")
    idx_in = nc.dram_tensor("idx16", [P, Lm], i16, kind="ExternalInput")
    dinv_in = nc.dram_tensor("dinv", [P, CNL], f32, kind="ExternalInput")
    wtab_in = nc.dram_tensor("wtab", [3, 64, 64], f32, kind="ExternalInput")
    btab_in = nc.dram_tensor("btab", [3, P, 64], f32, kind="ExternalInput")
    utab_in = nc.dram_tensor("utab", [3, P, 64], f32, kind="ExternalInput")
    a_in = nc.dram_tensor("amat", [NPAD, NCMT_SLOTS * P], bf16, kind="ExternalInput")
    lin1t_in = nc.dram_tensor("lin1t", [4 * P, HFC], f32, kind="ExternalInput")
    lin1b_in = nc.dram_tensor("lin1b", [HB, 2], f32, kind="ExternalInput")
    lin2t_in = nc.dram_tensor("lin2t", [HB, 2 * NCLS], f32, kind="ExternalInput")
    lin2b_in = nc.dram_tensor("lin2b", [BS, NCLS], f32, kind="ExternalInput")
    out_t = nc.dram_tensor("out", [BS, NCLS], f32, kind="ExternalOutput")
    ag_in = nc.dram_tensor("ag_in", [NL, FB], f8, kind="Internal")
    ag_out = nc.dram_tensor("ag_out", [NPAD, FB], f8, kind="Internal", addr_space="Shared")
    qag_in = nc.dram_tensor("qag_in", [NL, BS], f32, kind="Internal")
    qag_out = nc.dram_tensor("qag_out", [NPAD, BS], f32, kind="Internal", addr_space="Shared")

    RG = [list(range(NC8))]
    SGA = 8  # A-matrix staging slots

    with tile.TileContext(nc) as tc:
        with (
            tc.tile_pool(name="main", bufs=1) as pool,
            tc.tile_pool(name="psum", bufs=2, space="PSUM") as pp,
        ):
            h = pool.tile([P, CNL * FB], f32, tag="h")
            m_sb = pool.tile([P, CNL * FB], f32, tag="m_sb")
            acc = pool.tile([P, CNL * FB], f32, tag="acc")
            m_bf = pool.tile([P, CNL * FB], f8, tag="m_bf")
            gbuf = [pool.tile([P, SG * FB], f8, tag=f"g{i}", name=f"g{i}") for i in range(2)]
            abuf = [pool.tile([P, SGA * 512], bf16, tag=f"a{i}", name=f"a{i}") for i in range(2)]
            idx_sb = pool.tile([P, Lm], i16, tag="idx_sb")
            dinv_sb = pool.tile([P, CNL], f32, tag="dinv_sb")
            wt_sb = pool.tile([64, 3 * 64], f32, tag="wt_sb")
            b_sb = pool.tile([P, 3 * 64], f32, tag="b_sb")
            u_sb = pool.tile([P, 3 * 64], f32, tag="u_sb")
            ident = pool.tile([P, P], f32, tag="ident")
            qacc = pool.tile([P, CNL * BS], f32, tag="qacc")
            qc = pool.tile([P, CNL * BS], f32, tag="qc")
            qfull = pool.tile([P, (NPAD // P) * BS], f32, tag="qfull")
            qfull_bf = pool.tile([P, (NPAD // P) * BS], bf16, tag="qfull_bf")
            s_sb = pool.tile([P, NCMT_SLOTS * BS], f32, tag="s_sb")
            lin1t_sb = pool.tile([P, 4 * 2 * HB], f32, tag="lin1t_sb")
            lin1b_sb = pool.tile([HB, 2], f32, tag="lin1b_sb")
            lin2t_sb = pool.tile([HB, 2 * NCLS], f32, tag="lin2t_sb")
            lin2b_sb = pool.tile([BS, NCLS], f32, tag="lin2b_sb")
            z1_sb = pool.tile([HB, 2 * BS], f32, tag="z1_sb")
            z2_sb = pool.tile([BS, NCLS], f32, tag="z2_sb")
            sm_t = pool.tile([BS, NCLS], f32, tag="sm_t")
            sm_e = pool.tile([BS, NCLS], f32, tag="sm_e")
            sm_r = pool.tile([BS, 2], f32, tag="sm_r")

            nc.sync.dma_start(idx_sb[:], idx_in[:])
            nc.sync.dma_start(dinv_sb[:], dinv_in[:])
            for l in range(3):
                nc.sync.dma_start(wt_sb[:, l * 64 : (l + 1) * 64], wtab_in[l])
                nc.sync.dma_start(b_sb[:, l * 64 : (l + 1) * 64], btab_in[l])
                nc.sync.dma_start(u_sb[:, l * 64 : (l + 1) * 64], utab_in[l])
            nc.sync.dma_start(
                lin1t_sb[:].rearrange("p (k m) -> p k m", k=4),
                lin1t_in[:].rearrange("(k p) m -> p k m", p=P),
            )
            nc.sync.dma_start(lin1b_sb[:], lin1b_in[:])
            nc.sync.dma_start(lin2t_sb[:], lin2t_in[:])
            nc.sync.dma_start(lin2b_sb[:], lin2b_in[:])
            make_identity(nc, ident[:])

            h4 = h[:].rearrange("p (s b f) -> p s b f", b=BS, f=F)
            m4 = m_sb[:].rearrange("p (s b f) -> p s b f", b=BS, f=F)
            dinv_b = (
                dinv_sb[:].rearrange("p s -> p s () ()").to_broadcast([P, CNL, BS, F])
            )
            # x straight into h (host packs the local layout)
            nc.sync.dma_start(h4, x_in[:].rearrange("(s p) f -> p s f", p=P))

            for l in range(3):
                bl = b_sb[:, l * 64 : (l + 1) * 64]
                ul = u_sb[:, l * 64 : (l + 1) * 64]
                nc.vector.tensor_tensor(out=h4, in0=h4, in1=dinv_b, op=ALU.mult)
                for s in range(CNL):
                    for b in range(BS):
                        pt = pp.tile([64, P], f32, tag="pt")
                        nc.tensor.transpose(pt[:], h4[:, s, b, :], ident[:])
                        ht = pool.tile([64, P], f32, tag="ht")
                        nc.vector.tensor_copy(ht[:], pt[:])
                        pm = pp.tile([P, F], f32, tag="pm")
                        nc.tensor.matmul(
                            pm[:],
                            lhsT=ht[:],
                            rhs=wt_sb[:, l * 64 : (l + 1) * 64],
                            start=True,
                            stop=True,
                        )
                        nc.scalar.activation(m4[:, s, b, :], pm[:], AF.Copy)
                nc.vector.tensor_copy(m_bf[:], m_sb[:])
                nc.sync.dma_start(
                    ag_in[:].rearrange("(s p) f -> p s f", p=P),
                    m_bf[:].rearrange("p (s f) -> p s f", f=FB),
                )
                nc.gpsimd.collective_compute(
                    "AllGather",
                    ALU.bypass,
                    replica_groups=RG,
                    ins=[ag_in[:]],
                    outs=[ag_out[:]],
                )
                # self-loop round: acc = m (local, fp32)
                nc.vector.tensor_copy(acc[:], m_sb[:])
                for g, runs in enumerate(groups):
                    gt = gbuf[g % 2]
                    nc.gpsimd.dma_gather(
                        out_ap=gt[:].rearrange("p (s f) -> p s f", f=FB),
                        in_ap=ag_out[:],
                        idxs_ap=idx_sb[:, g * SG * 8 : (g + 1) * SG * 8],
                        num_idxs=SG * P,
                        num_idxs_reg=SG * P,
                        elem_size=FB,
                        single_packet=False,
                    )
                    for (ss, a0, ln) in runs:
                        nc.vector.tensor_tensor(
                            out=acc[:, a0 * FB : (a0 + ln) * FB],
                            in0=acc[:, a0 * FB : (a0 + ln) * FB],
                            in1=gt[:, ss * FB : (ss + ln) * FB],
                            op=ALU.add,
                        )
                acc4 = acc[:].rearrange("p (s b f) -> p s b f", b=BS, f=F)
                nc.vector.tensor_tensor(out=h4, in0=acc4, in1=dinv_b, op=ALU.mult)
                nc.vector.tensor_tensor(
                    out=h4,
                    in0=h4,
                    in1=bl.rearrange("p f -> p () () f").to_broadcast([P, CNL, BS, F]),
                    op=ALU.add,
                )
                nc.scalar.activation(h[:], h[:], AF.Relu)
                nc.vector.tensor_tensor(
                    out=m4,
                    in0=h4,
                    in1=ul.rearrange("p f -> p () () f").to_broadcast([P, CNL, BS, F]),
                    op=ALU.mult,
                )
                if l == 0:
                    nc.vector.reduce_sum(qacc[:], m4, axis=AX.X)
                else:
                    nc.vector.reduce_sum(qc[:], m4, axis=AX.X)
                    nc.vector.tensor_tensor(
                        out=qacc[:], in0=qacc[:], in1=qc[:], op=ALU.add
                    )

            # ---- pathway: s = A^T q via PE ----
            nc.sync.dma_start(
                qag_in[:].rearrange("(s p) b -> p s b", p=P),
                qacc[:].rearrange("p (s b) -> p s b", b=BS),
            )
            nc.gpsimd.collective_compute(
                "AllGather", ALU.bypass, replica_groups=RG,
                ins=[qag_in[:]], outs=[qag_out[:]],
            )
            nc.sync.dma_start(
                qfull[:].rearrange("p (s b) -> p s b", b=BS),
                qag_out[:].rearrange("(s p) b -> p s b", p=P),
            )
            nc.vector.tensor_copy(qfull_bf[:], qfull[:])
            qf3 = qfull_bf[:].rearrange("p (s b) -> p s b", b=BS)
            a_view = a_in[:].rearrange("(s p) c -> p s c", p=P)
            NSLOT = NPAD // P  # 120
            s_ps_all = pp.tile([P, 4 * BS], f32, tag="s_ps", bufs=1, name="s_ps_all")
            s_ps = [s_ps_all[:, g * BS : (g + 1) * BS] for g in range(4)]
            for sc in range(NSLOT):
                if sc % SGA == 0:
                    ab = abuf[(sc // SGA) % 2]
                    nc.sync.dma_start(
                        ab[:].rearrange("p (k c) -> p k c", k=SGA),
                        a_view[:, sc : sc + SGA, :],
                    )
                ab = abuf[(sc // SGA) % 2]
                a3 = ab[:].rearrange("p (k c) -> p k c", k=SGA)
                for gp in range(4):
                    nc.tensor.matmul(
                        s_ps[gp],
                        lhsT=a3[:, sc % SGA, gp * P : (gp + 1) * P],
                        rhs=qf3[:, sc, :],
                        start=(sc == 0),
                        stop=(sc == NSLOT - 1),
                        skip_group_check=True,
                    )
            s3 = s_sb[:].rearrange("p (k b) -> p k b", b=BS)
            for gp in range(4):
                nc.vector.tensor_copy(s3[:, gp, :], s_ps[gp])

            # ---- head (all 8 batches at once) ----
            for j, m0 in enumerate((0, HB)):
                pz = pp.tile([HB, BS], f32, tag="pz", bufs=1)
                for kc in range(4):
                    nc.tensor.matmul(
                        pz[:],
                        lhsT=lin1t_sb[:].rearrange("p (k m) -> p k m", k=4)[
                            :, kc, m0 : m0 + HB
                        ],
                        rhs=s3[:, kc, :],
                        start=(kc == 0),
                        stop=(kc == 3),
                    )
                nc.scalar.activation(
                    z1_sb[:].rearrange("p (j b) -> p j b", b=BS)[:, j, :],
                    pz[:],
                    AF.Relu,
                    bias=lin1b_sb[:, j : j + 1],
                )
            pz2 = pp.tile([BS, NCLS], f32, tag="pz2", bufs=1)
            z13 = z1_sb[:].rearrange("p (j b) -> p j b", b=BS)
            for j in range(2):
                nc.tensor.matmul(
                    pz2[:],
                    lhsT=z13[:, j, :],
                    rhs=lin2t_sb[:, j * NCLS : (j + 1) * NCLS],
                    start=(j == 0),
                    stop=(j == 1),
                )
            nc.vector.tensor_tensor(out=z2_sb[:], in0=pz2[:], in1=lin2b_sb[:], op=ALU.add)
            nc.vector.reduce_max(sm_r[:, 0:1], z2_sb[:], axis=AX.X)
            nc.vector.tensor_tensor(
                out=sm_t[:],
                in0=z2_sb[:],
                in1=sm_r[:, 0:1].to_broadcast([BS, NCLS]),
                op=ALU.subtract,
            )
            nc.scalar.activation(sm_e[:], sm_t[:], AF.Exp)
            nc.vector.reduce_sum(sm_r[:, 1:2], sm_e[:], axis=AX.X)
            nc.scalar.activation(sm_r[:, 1:2], sm_r[:, 1:2], AF.Ln)
            nc.vector.tensor_tensor(
                out=sm_t[:],
                in0=sm_t[:],
                in1=sm_r[:, 1:2].to_broadcast([BS, NCLS]),
                op=ALU.subtract,
            )
            nc.sync.dma_start(out_t[:], sm_t[:])

    nc.compile()
    return nc


def _make_in_maps(inputs, prep):
    import ml_dtypes

    x = np.asarray(inputs["x"], np.float32)
    perm = prep["perm"]
    wtab = np.stack(
        [np.asarray(inputs[f"W{i}"], np.float32).T.copy() for i in (1, 2, 3)]
    )
    btab = np.stack(
        [np.tile(np.asarray(inputs[f"b{i}"], np.float32), (P, 1)) for i in (1, 2, 3)]
    )
    utab = np.stack([np.tile(u, (P, 1)) for u in prep["u"]])
    amat = prep["A"].astype(ml_dtypes.bfloat16)
    lin1t = np.zeros((4 * P, HFC), np.float32)
    lin1t[:NCMT] = prep["lin1_eff"].T
    lin1b = prep["lin1_b_eff"].reshape(2, HB).T.copy()
    lin2 = np.asarray(inputs["lin2_w"], np.float32)
    lin2t = np.concatenate([lin2.T[:HB], lin2.T[HB:]], axis=1)  # [100, 4]
    lin2b = np.tile(np.asarray(inputs["lin2_b"], np.float32).reshape(1, NCLS), (BS, 1))
    shared = dict(
        wtab=wtab, btab=btab, utab=utab, amat=amat,
        lin1t=lin1t, lin1b=lin1b, lin2t=lin2t, lin2b=lin2b,
    )
    # per-core x_local [NL, FB] and dinv/idx
    perm_pad = np.zeros(NPAD, np.int64)
    perm_pad[:N] = perm
    valid = np.zeros(NPAD, np.float32)
    valid[:N] = 1.0
    maps = []
    for c in range(NC8):
        rs = np.arange(NL) * NC8 + c
        olds = perm_pad[rs]
        xl = x[:, olds, :].transpose(1, 0, 2).reshape(NL, FB).copy()
        xl *= valid[rs][:, None]
        maps.append(
            dict(
                shared,
                x=xl,
                idx16=prep["idx16"][c],
                dinv=prep["dinv_loc"][c],
            )
        )
    return maps


_TRACE = {"trace": False, "last_ns": None}


def kernel(**inputs):
    from concourse.bass_utils import run_bass_kernel_spmd

    prep = host_prep(
        np.asarray(inputs["edge_index"]),
        np.asarray(inputs["row"]),
        np.asarray(inputs["col"]),
        np.asarray(inputs["fc_w"]),
        np.asarray(inputs["fc_b"]),
        np.asarray(inputs["lin1_w"]),
        np.asarray(inputs["lin1_b"]),
    )
    nc = build_bass(prep)
    in_maps = _make_in_maps(inputs, prep)
    res = run_bass_kernel_spmd(
        nc, in_maps, core_ids=list(range(NC8)), trace=_TRACE["trace"]
    )
    _TRACE["last_ns"] = res.exec_time_ns
    return res.results[0]["out"].reshape(BS, NCLS).astype(np.float32)


if __name__ == "__main__":
    import reference

    inputs = {k: np.asarray(v) for k, v in reference.setup_inputs().items()}
    expected = np.asarray(reference.reference(**inputs))
    prep = host_prep(
        inputs["edge_index"], inputs["row"], inputs["col"],
        inputs["fc_w"], inputs["fc_b"], inputs["lin1_w"], inputs["lin1_b"],
    )
    for bf in (False, True):
        got = golden(inputs, prep, bf16=bf)
        err = np.abs(got - expected).max()
        rel = err / np.abs(expected).max()
        print(f"golden(bf16={bf}) abs {err:.3e} rel {rel:.3e}")
